# revision 12
# baseline (speedup 1.0000x reference)
"""Multi-head attention Trainium2 Bass kernel (v2).

Problem: B=2, S=2048, D=1024, H=16, HS=64.
Sharding: tensor-parallel over heads — each of 8 cores computes 2 heads
(128 contiguous output-feature columns) for both batches; host concatenates.

v2 design (vs the v1 phase-separated kernel):
  * The scalar-engine exp stream is the roofline: 16.8M elements/core at
    1 elem/lane/cycle @1.2GHz = ~110us, ~147us with the 352-cycle
    per-ACTIVATE overhead at N=1024.  Everything else hides under it.
  * q-blocks of 512: per (batch, qblock, key-chunk) one PSUM tile
    [128 keys, 1024] holds both heads' sims side by side, so one ACTIVATE
    covers both heads.  PSUM: sim 2x2 banks + pvp 2x1 + proj 2x1 = 8.
  * The two sim matmuls (K=64 each) go to disjoint PE row groups
    (h0 rows 0-63, h1 rows 64-127) emitted back-to-back so they can run
    concurrently in the array.
  * No bias matmuls at all: bK is dropped (constant per q column ->
    cancels in softmax), bQ is added by the DVE during the Q psum->sbuf
    copy (per-partition scalar add), bV is applied by the host after the
    final divide (P@(v+b)/P@1 = P@v/P@1 + b).
  * Projections are emitted after the attention loops at lower scheduler
    priority; the Tile scheduler runs them in PE gaps under the exp
    stream.  Only the first q-block's K/V/Q prefix runs up front.
  * V' is token-major [tok, 64v+1] per head; the softmax-denominator ones
    columns are pre-memset into SBUF (stride-65 pattern) instead of being
    produced by a bias matmul.
"""

import sys

sys.path.insert(0, "/opt/trn_rl_repo")

import ml_dtypes
import numpy as np

import concourse.bass as bass
import concourse.mybir as mybir
import concourse.tile as tile
from concourse import bacc
from concourse import bass_utils

B, S, D = 2, 2048, 1024
H, HS = 16, 64
NCORES = 8
NTOK = B * S                  # 4096
FPC = (H // NCORES) * HS      # 128 output-feature cols per core (2 heads)
TT = 512                      # token tile for projections
NTT = NTOK // TT              # 8
NCH = D // 128                # 8 contraction chunks
QB = 512                      # q block width
NQB = S // QB                 # 4 q blocks per batch
KT = 128                      # key chunk in attention
NKT = S // KT                 # 16
VW = 2 * (HS + 1)             # 130: [V_h0 | 1 | V_h1 | 1] columns per chunk

F32 = mybir.dt.float32
BF16 = mybir.dt.bfloat16
I16 = mybir.dt.int16

# Schraudolph exp on the DVE, int16 form: bf16bits(exp(sim/8)) ~=
# round(sim * 2^7*log2e/8 + (127*2^7 - C16 - 0.5)).  C16 = 366393/65536
# (the standard rms-optimal int32 constant scaled to the bf16 grid); the
# -0.5 converts round-to-nearest into the truncation the constant was
# tuned for.  Every 4th key-chunk uses this path, freeing the scalar
# engine; measured end-to-end rms error ~5.3e-3 (gate 2e-2).
SCH_A = 128.0 * 1.4426950408889634 / 8.0
SCH_B = 127.0 * 128.0 - 366393.0 / 65536.0 - 0.5
SCH_EVERY = 4

_NC_CACHE = {}


def build_nc():
    nc = bacc.Bacc("TRN2", target_bir_lowering=False, debug=False, num_devices=NCORES)
    xt = nc.dram_tensor("xt", [D, NTOK], BF16, kind="ExternalInput").ap()
    # weights ship pre-swizzled to the SBUF layout [128, chunk*feat]
    wq = nc.dram_tensor("wq", [128, NCH * FPC], BF16, kind="ExternalInput").ap()
    wk = nc.dram_tensor("wk", [128, NCH * FPC], BF16, kind="ExternalInput").ap()
    wv = nc.dram_tensor("wv", [128, NCH * FPC], BF16, kind="ExternalInput").ap()
    bq = nc.dram_tensor("bq", [FPC, 1], F32, kind="ExternalInput").ap()
    out = nc.dram_tensor("out", [2 * (HS + 1), NTOK], F32, kind="ExternalOutput").ap()

    with tile.TileContext(nc) as tc:
        with (
            tc.tile_pool(name="persist", bufs=1) as pp,
            tc.tile_pool(name="work", bufs=2) as wkp,
            tc.tile_pool(name="psA", bufs=2, space="PSUM") as psA,
            tc.tile_pool(name="psB", bufs=2, space="PSUM") as psB,
        ):
            # ---------------- persistent SBUF ------------------------------
            xtc = [pp.tile([128, NTOK], BF16, name=f"xt_{c}") for c in range(NCH)]
            wq_sb = pp.tile([128, NCH * FPC], BF16)
            wk_sb = pp.tile([128, NCH * FPC], BF16)
            wv_sb = pp.tile([128, NCH * FPC], BF16)
            bq_sb = pp.tile([128, 1], F32)
            qt_sb = pp.tile([128, NTOK], BF16)   # Q^T + bQ: [feat(2 heads), tok]
            kt_sb = pp.tile([128, NTOK], BF16)   # K^T (no bias)
            vp_sb = pp.tile([128, (NTOK // 128) * VW], BF16)  # V' [tok128 chunks, 65+65]

            # DMA order = first-exp critical path: K weights, then t0 X
            # slices, then the rest in increasing-deadline order with
            # progressively larger (faster) transfers.
            nc.sync.dma_start(bq_sb[:], bq[:, :])
            nc.sync.dma_start(wk_sb[:], wk[:, :])
            for c in range(NCH):
                nc.sync.dma_start(xtc[c][:, 0:TT], xt[c * 128 : (c + 1) * 128, 0:TT])
            nc.sync.dma_start(wq_sb[:], wq[:, :])
            nc.sync.dma_start(wv_sb[:], wv[:, :])
            for c in range(NCH):
                nc.sync.dma_start(xtc[c][:, TT : 2 * TT], xt[c * 128 : (c + 1) * 128, TT : 2 * TT])
            for c in range(NCH):
                nc.sync.dma_start(xtc[c][:, 2 * TT : 4 * TT], xt[c * 128 : (c + 1) * 128, 2 * TT : 4 * TT])
            for c in range(NCH):
                nc.sync.dma_start(xtc[c][:, S : 2 * S], xt[c * 128 : (c + 1) * 128, S : 2 * S])

            # Dummy matmuls on never-written SBUF warm the PE clock gate
            # (HAM) during the DMA prefix so the real prefix runs at 2.4GHz.
            warm_in = pp.tile([64, TT], BF16, name="warm_in")
            nc.vector.memset(warm_in[:], 0.0)
            for w in range(10):
                wps = psA.tile([128, TT], F32, name=f"warm_{w}", tag="prj", padded_shape=[128, TT])
                nc.tensor.matmul(wps[:], warm_in[:, 0:128], warm_in[:], start=True, stop=True)

            # Softmax-denominator ones columns: cols 64, 129, 194, ... (64+65k).
            nc.vector.memset(vp_sb[:, HS :: HS + 1], 1.0)

            # ---------------- projection building blocks -------------------
            def proj_qk(t, which):
                """Project Q or K for t-tile t (512 tokens), feature-major."""
                tsl = slice(t * TT, (t + 1) * TT)
                w_sb, dst = (wq_sb, qt_sb) if which == "q" else (wk_sb, kt_sb)
                pj = psA.tile([128, TT], F32, name=f"pj_{which}_{t}", tag="prj", padded_shape=[128, TT])
                for c in range(NCH):
                    nc.tensor.matmul(
                        pj[:], w_sb[:, c * FPC : (c + 1) * FPC], xtc[c][:, tsl],
                        start=(c == 0), stop=(c == NCH - 1),
                    )
                if which == "q":
                    nc.vector.tensor_scalar_add(dst[:, tsl], pj[:], bq_sb[:])
                else:
                    nc.vector.tensor_copy(dst[:, tsl], pj[:])

            def proj_v(ch):
                """Project V' token-major for 128-token chunk ch."""
                psv = psA.tile([128, FPC], F32, name=f"pv_{ch}", tag="prj", padded_shape=[128, TT])
                for c in range(NCH):
                    nc.tensor.matmul(
                        psv[:], xtc[c][:, ch * 128 : (ch + 1) * 128],
                        wv_sb[:, c * FPC : (c + 1) * FPC],
                        start=(c == 0), stop=(c == NCH - 1),
                    )
                base = ch * VW
                for h in range(2):
                    nc.vector.tensor_copy(
                        vp_sb[:, base + h * (HS + 1) : base + h * (HS + 1) + HS],
                        psv[:, h * HS : (h + 1) * HS],
                    )

            # ---------------- attention ------------------------------------
            def attn_qblock(b, qb):
                qsl = slice(b * S + qb * QB, b * S + (qb + 1) * QB)
                pvp = [
                    psB.tile([HS + 1, QB], F32, name=f"pvp_{b}_{qb}_{h}", tag="pvp", padded_shape=[128, QB])
                    for h in range(2)
                ]
                for kt in range(NKT):
                    ksl = slice(b * S + kt * KT, b * S + (kt + 1) * KT)
                    ch = b * NKT + kt
                    simt = psA.tile([128, 2 * QB], F32, name=f"sim_{b}_{qb}_{kt}", tag="sim", padded_shape=[128, 2 * QB])
                    # two K=64 matmuls into disjoint PE row groups, adjacent
                    # in the queue so they can run concurrently
                    for h in range(2):
                        hp = h * HS
                        nc.tensor.matmul(
                            simt[:, h * QB : (h + 1) * QB],
                            kt_sb[hp : hp + HS, ksl],
                            qt_sb[hp : hp + HS, qsl],
                            start=True, stop=True,
                            tile_position=(hp, 0),
                        )
                    pt = wkp.tile([128, 2 * QB], BF16, name=f"pt_{b}_{qb}_{kt}", tag="pt", bufs=3)
                    if kt % SCH_EVERY == 1:
                        # Schraudolph exp on the DVE (int16 bit-trick),
                        # freeing the scalar engine for this chunk.
                        nc.vector.tensor_scalar(
                            pt.bitcast(I16)[:], simt[:], SCH_A, SCH_B,
                            op0=mybir.AluOpType.mult, op1=mybir.AluOpType.add,
                        )
                    else:
                        nc.scalar.activation(pt[:], simt[:], mybir.ActivationFunctionType.Exp, scale=1.0 / np.sqrt(HS))
                    for h in range(2):
                        nc.tensor.matmul(
                            pvp[h][:],
                            vp_sb[:, ch * VW + h * (HS + 1) : ch * VW + (h + 1) * (HS + 1)],
                            pt[:, h * QB : (h + 1) * QB],
                            start=(kt == 0), stop=(kt == NKT - 1),
                        )
                for h in range(2):
                    ot = wkp.tile([HS + 1, QB], F32, name=f"ot_{b}_{qb}_{h}", tag="ot", bufs=4)
                    nc.vector.tensor_copy(ot[:], pvp[h][:])
                    nc.sync.dma_start(
                        out[h * (HS + 1) : (h + 1) * (HS + 1), qsl], ot[:]
                    )

            # ---------------- schedule -------------------------------------
            # Projections must be EMITTED before their consumers (Tile deps
            # follow program order), but everything past the minimal prefix
            # is wrapped in a low-priority region (negative high_priority
            # offset) so the scheduler runs it in PE gaps under the exp
            # stream instead of ahead of attention work.
            LOW = -(1 << 20)

            proj_qk(0, "k")
            proj_qk(0, "q")
            proj_v(0)
            proj_v(1)
            with tc.high_priority(offset=LOW):
                for ch in range(2, 4):
                    proj_v(ch)
                for t in range(1, 4):
                    proj_qk(t, "k")
                    for ch in range(4 * t, 4 * t + 4):
                        proj_v(ch)
            attn_qblock(0, 0)
            with tc.high_priority(offset=LOW):
                proj_qk(1, "q")
            attn_qblock(0, 1)
            with tc.high_priority(offset=LOW):
                proj_qk(2, "q")
                for t in range(4, 8):
                    proj_qk(t, "k")
                    for ch in range(4 * t, 4 * t + 4):
                        proj_v(ch)
                proj_qk(4, "q")
            attn_qblock(0, 2)
            with tc.high_priority(offset=LOW):
                proj_qk(3, "q")
            attn_qblock(0, 3)
            attn_qblock(1, 0)
            with tc.high_priority(offset=LOW):
                proj_qk(5, "q")
            attn_qblock(1, 1)
            with tc.high_priority(offset=LOW):
                proj_qk(6, "q")
            attn_qblock(1, 2)
            with tc.high_priority(offset=LOW):
                proj_qk(7, "q")
            attn_qblock(1, 3)

    nc.compile()
    return nc


def get_nc():
    if "nc" not in _NC_CACHE:
        _NC_CACHE["nc"] = build_nc()
    return _NC_CACHE["nc"]


def make_in_maps(seq_input, WQ, bQ, WK, bK, WV, bV):
    x = np.asarray(seq_input, dtype=np.float32).reshape(NTOK, D)
    xt = np.ascontiguousarray(x.T).astype(ml_dtypes.bfloat16)

    def swizzle(w):
        # [D, FPC] -> [128, NCH*FPC]: row p, col c*FPC+j = w[c*128+p, j]
        return np.ascontiguousarray(
            w.reshape(NCH, 128, FPC).transpose(1, 0, 2).reshape(128, NCH * FPC)
        ).astype(ml_dtypes.bfloat16)

    in_maps = []
    for c in range(NCORES):
        lo, hi = c * FPC, (c + 1) * FPC
        in_maps.append(
            {
                "xt": xt,
                "wq": swizzle(WQ[:, lo:hi]),
                "wk": swizzle(WK[:, lo:hi]),
                "wv": swizzle(WV[:, lo:hi]),
                "bq": np.ascontiguousarray(bQ[lo:hi]).reshape(FPC, 1).astype(np.float32),
            }
        )
    return in_maps


def assemble(res, bV):
    """res -> full [B, S, H*HS] output; applies the host-side V bias."""
    parts = []
    for c in range(NCORES):
        o = res.results[c]["out"]  # [130, 4096] unnormalized, feature-major
        lo = c * FPC
        for h in range(2):
            num = o[h * (HS + 1) : h * (HS + 1) + HS, :]      # [64, 4096]
            den = o[h * (HS + 1) + HS, :]                     # [4096]
            parts.append((num / den).T + bV[lo + h * HS : lo + (h + 1) * HS])
    full = np.concatenate(parts, axis=1)  # [4096, 1024]
    return full.reshape(B, S, H * HS)


def run(in_maps, trace=False):
    nc = get_nc()
    return bass_utils.run_bass_kernel_spmd(nc, in_maps, core_ids=list(range(NCORES)), trace=trace)


def kernel(seq_input, WQ, bQ, WK, bK, WV, bV):
    in_maps = make_in_maps(
        np.asarray(seq_input, np.float32),
        np.asarray(WQ, np.float32), np.asarray(bQ, np.float32),
        np.asarray(WK, np.float32), np.asarray(bK, np.float32),
        np.asarray(WV, np.float32), np.asarray(bV, np.float32),
    )
    res = run(in_maps)
    return assemble(res, np.asarray(bV, np.float32))


# revision 13
# speedup vs baseline: 1.0307x; 1.0307x over previous
"""Multi-head attention Trainium2 Bass kernel (v2).

Problem: B=2, S=2048, D=1024, H=16, HS=64.
Sharding: tensor-parallel over heads — each of 8 cores computes 2 heads
(128 contiguous output-feature columns) for both batches; host concatenates.

v2 design (vs the v1 phase-separated kernel):
  * The scalar-engine exp stream is the roofline: 16.8M elements/core at
    1 elem/lane/cycle @1.2GHz = ~110us, ~147us with the 352-cycle
    per-ACTIVATE overhead at N=1024.  Everything else hides under it.
  * q-blocks of 512: per (batch, qblock, key-chunk) one PSUM tile
    [128 keys, 1024] holds both heads' sims side by side, so one ACTIVATE
    covers both heads.  PSUM: sim 2x2 banks + pvp 2x1 + proj 2x1 = 8.
  * The two sim matmuls (K=64 each) go to disjoint PE row groups
    (h0 rows 0-63, h1 rows 64-127) emitted back-to-back so they can run
    concurrently in the array.
  * No bias matmuls at all: bK is dropped (constant per q column ->
    cancels in softmax), bQ is added by the DVE during the Q psum->sbuf
    copy (per-partition scalar add), bV is applied by the host after the
    final divide (P@(v+b)/P@1 = P@v/P@1 + b).
  * Projections are emitted after the attention loops at lower scheduler
    priority; the Tile scheduler runs them in PE gaps under the exp
    stream.  Only the first q-block's K/V/Q prefix runs up front.
  * V' is token-major [tok, 64v+1] per head; the softmax-denominator ones
    columns are pre-memset into SBUF (stride-65 pattern) instead of being
    produced by a bias matmul.
"""

import sys

sys.path.insert(0, "/opt/trn_rl_repo")

import ml_dtypes
import numpy as np

import concourse.bass as bass
import concourse.mybir as mybir
import concourse.tile as tile
from concourse import bacc
from concourse import bass_utils

B, S, D = 2, 2048, 1024
H, HS = 16, 64
NCORES = 8
NTOK = B * S                  # 4096
FPC = (H // NCORES) * HS      # 128 output-feature cols per core (2 heads)
TT = 512                      # token tile for projections
NTT = NTOK // TT              # 8
NCH = D // 128                # 8 contraction chunks
QB = 512                      # q block width
NQB = S // QB                 # 4 q blocks per batch
KT = 128                      # key chunk in attention
NKT = S // KT                 # 16
VW = 2 * (HS + 1)             # 130: [V_h0 | 1 | V_h1 | 1] columns per chunk

F32 = mybir.dt.float32
BF16 = mybir.dt.bfloat16
I16 = mybir.dt.int16

# Schraudolph exp on the DVE, int16 form: bf16bits(exp(sim/8)) ~=
# round(sim * 2^7*log2e/8 + (127*2^7 - C16 - 0.5)).  C16 = 366393/65536
# (the standard rms-optimal int32 constant scaled to the bf16 grid); the
# -0.5 converts round-to-nearest into the truncation the constant was
# tuned for.  Every 4th key-chunk uses this path, freeing the scalar
# engine; measured end-to-end rms error ~5.3e-3 (gate 2e-2).
SCH_A = 128.0 * 1.4426950408889634 / 8.0
SCH_B = 127.0 * 128.0 - 366393.0 / 65536.0 - 0.5
# Schraudolph currently disabled: with PE ~= ACT per key-chunk, moving exp
# to the DVE fragments the stream (strict-FIFO head-of-line blocking)
# without lowering the wall clock.  Set to 4 to re-enable (rms ~5.9e-3).
SCH_EVERY = 10**9

_NC_CACHE = {}


def build_nc():
    nc = bacc.Bacc("TRN2", target_bir_lowering=False, debug=False, num_devices=NCORES)
    xt = nc.dram_tensor("xt", [D, NTOK], BF16, kind="ExternalInput").ap()
    # weights ship pre-swizzled to the SBUF layout [128, chunk*feat]
    wq = nc.dram_tensor("wq", [128, NCH * FPC], BF16, kind="ExternalInput").ap()
    wk = nc.dram_tensor("wk", [128, NCH * FPC], BF16, kind="ExternalInput").ap()
    wv = nc.dram_tensor("wv", [128, NCH * FPC], BF16, kind="ExternalInput").ap()
    bq = nc.dram_tensor("bq", [FPC, 1], F32, kind="ExternalInput").ap()
    out = nc.dram_tensor("out", [2 * (HS + 1), NTOK], F32, kind="ExternalOutput").ap()

    with tile.TileContext(nc) as tc:
        with (
            tc.tile_pool(name="persist", bufs=1) as pp,
            tc.tile_pool(name="work", bufs=2) as wkp,
            tc.tile_pool(name="psA", bufs=2, space="PSUM") as psA,
            tc.tile_pool(name="psB", bufs=2, space="PSUM") as psB,
        ):
            # ---------------- persistent SBUF ------------------------------
            xtc = [pp.tile([128, NTOK], BF16, name=f"xt_{c}") for c in range(NCH)]
            wq_sb = pp.tile([128, NCH * FPC], BF16)
            wk_sb = pp.tile([128, NCH * FPC], BF16)
            wv_sb = pp.tile([128, NCH * FPC], BF16)
            bq_sb = pp.tile([128, 1], F32)
            qt_sb = pp.tile([128, NTOK], BF16)   # Q^T + bQ: [feat(2 heads), tok]
            kt_sb = pp.tile([128, NTOK], BF16)   # K^T (no bias)
            vp_sb = pp.tile([128, (NTOK // 128) * VW], BF16)  # V' [tok128 chunks, 65+65]

            # DMA order = first-exp critical path: K weights, then t0 X
            # slices, then the rest in increasing-deadline order with
            # progressively larger (faster) transfers.
            nc.sync.dma_start(bq_sb[:], bq[:, :])
            nc.sync.dma_start(wk_sb[:], wk[:, :])
            for c in range(NCH):
                nc.sync.dma_start(xtc[c][:, 0:TT], xt[c * 128 : (c + 1) * 128, 0:TT])
            nc.sync.dma_start(wq_sb[:], wq[:, :])
            nc.sync.dma_start(wv_sb[:], wv[:, :])
            for c in range(NCH):
                nc.sync.dma_start(xtc[c][:, TT : 2 * TT], xt[c * 128 : (c + 1) * 128, TT : 2 * TT])
            for c in range(NCH):
                nc.sync.dma_start(xtc[c][:, 2 * TT : 4 * TT], xt[c * 128 : (c + 1) * 128, 2 * TT : 4 * TT])
            for c in range(NCH):
                nc.sync.dma_start(xtc[c][:, S : 2 * S], xt[c * 128 : (c + 1) * 128, S : 2 * S])

            # Dummy matmuls on never-written SBUF warm the PE clock gate
            # (HAM) during the DMA prefix so the real prefix runs at 2.4GHz.
            warm_in = pp.tile([64, TT], BF16, name="warm_in")
            nc.vector.memset(warm_in[:], 0.0)
            for w in range(10):
                wps = psA.tile([128, TT], F32, name=f"warm_{w}", tag="prj", padded_shape=[128, TT])
                nc.tensor.matmul(wps[:], warm_in[:, 0:128], warm_in[:], start=True, stop=True)

            # Softmax-denominator ones columns: cols 64, 129, 194, ... (64+65k).
            nc.vector.memset(vp_sb[:, HS :: HS + 1], 1.0)

            # ---------------- projection building blocks -------------------
            def proj_qk(t, which):
                """Project Q or K for t-tile t (512 tokens), feature-major."""
                tsl = slice(t * TT, (t + 1) * TT)
                w_sb, dst = (wq_sb, qt_sb) if which == "q" else (wk_sb, kt_sb)
                pj = psA.tile([128, TT], F32, name=f"pj_{which}_{t}", tag="prj", padded_shape=[128, TT])
                for c in range(NCH):
                    nc.tensor.matmul(
                        pj[:], w_sb[:, c * FPC : (c + 1) * FPC], xtc[c][:, tsl],
                        start=(c == 0), stop=(c == NCH - 1),
                    )
                if which == "q":
                    nc.vector.tensor_scalar_add(dst[:, tsl], pj[:], bq_sb[:])
                else:
                    nc.vector.tensor_copy(dst[:, tsl], pj[:])

            def proj_v(ch):
                """Project V' token-major for 128-token chunk ch."""
                psv = psA.tile([128, FPC], F32, name=f"pv_{ch}", tag="prj", padded_shape=[128, TT])
                for c in range(NCH):
                    nc.tensor.matmul(
                        psv[:], xtc[c][:, ch * 128 : (ch + 1) * 128],
                        wv_sb[:, c * FPC : (c + 1) * FPC],
                        start=(c == 0), stop=(c == NCH - 1),
                    )
                base = ch * VW
                for h in range(2):
                    nc.vector.tensor_copy(
                        vp_sb[:, base + h * (HS + 1) : base + h * (HS + 1) + HS],
                        psv[:, h * HS : (h + 1) * HS],
                    )

            # ---------------- attention ------------------------------------
            def attn_qblock(b, qb):
                qsl = slice(b * S + qb * QB, b * S + (qb + 1) * QB)
                pvp = [
                    psB.tile([HS + 1, QB], F32, name=f"pvp_{b}_{qb}_{h}", tag="pvp", padded_shape=[128, QB])
                    for h in range(2)
                ]
                for kt in range(NKT):
                    ksl = slice(b * S + kt * KT, b * S + (kt + 1) * KT)
                    ch = b * NKT + kt
                    simt = psA.tile([128, 2 * QB], F32, name=f"sim_{b}_{qb}_{kt}", tag="sim", padded_shape=[128, 2 * QB])
                    # two K=64 matmuls into disjoint PE row groups, adjacent
                    # in the queue so they can run concurrently
                    for h in range(2):
                        hp = h * HS
                        nc.tensor.matmul(
                            simt[:, h * QB : (h + 1) * QB],
                            kt_sb[hp : hp + HS, ksl],
                            qt_sb[hp : hp + HS, qsl],
                            start=True, stop=True,
                            tile_position=(hp, 0),
                        )
                    pt = wkp.tile([128, 2 * QB], BF16, name=f"pt_{b}_{qb}_{kt}", tag="pt", bufs=3)
                    if kt % SCH_EVERY == 1:
                        # Schraudolph exp on the DVE (int16 bit-trick),
                        # freeing the scalar engine for this chunk.
                        nc.vector.tensor_scalar(
                            pt.bitcast(I16)[:], simt[:], SCH_A, SCH_B,
                            op0=mybir.AluOpType.mult, op1=mybir.AluOpType.add,
                        )
                    else:
                        nc.scalar.activation(pt[:], simt[:], mybir.ActivationFunctionType.Exp, scale=1.0 / np.sqrt(HS))
                    for h in range(2):
                        nc.tensor.matmul(
                            pvp[h][:],
                            vp_sb[:, ch * VW + h * (HS + 1) : ch * VW + (h + 1) * (HS + 1)],
                            pt[:, h * QB : (h + 1) * QB],
                            start=(kt == 0), stop=(kt == NKT - 1),
                        )
                for h in range(2):
                    ot = wkp.tile([HS + 1, QB], F32, name=f"ot_{b}_{qb}_{h}", tag="ot", bufs=4)
                    nc.vector.tensor_copy(ot[:], pvp[h][:])
                    nc.sync.dma_start(
                        out[h * (HS + 1) : (h + 1) * (HS + 1), qsl], ot[:]
                    )

            # ---------------- schedule -------------------------------------
            # Projections must be EMITTED before their consumers (Tile deps
            # follow program order), but everything past the minimal prefix
            # is wrapped in a low-priority region (negative high_priority
            # offset) so the scheduler runs it in PE gaps under the exp
            # stream instead of ahead of attention work.
            LOW = -(1 << 20)

            proj_qk(0, "k")
            proj_qk(0, "q")
            proj_v(0)
            proj_v(1)
            with tc.high_priority(offset=LOW):
                for ch in range(2, 4):
                    proj_v(ch)
                for t in range(1, 4):
                    proj_qk(t, "k")
                    for ch in range(4 * t, 4 * t + 4):
                        proj_v(ch)
            attn_qblock(0, 0)
            with tc.high_priority(offset=LOW):
                proj_qk(1, "q")
            attn_qblock(0, 1)
            with tc.high_priority(offset=LOW):
                proj_qk(2, "q")
                for t in range(4, 8):
                    proj_qk(t, "k")
                    for ch in range(4 * t, 4 * t + 4):
                        proj_v(ch)
                proj_qk(4, "q")
            attn_qblock(0, 2)
            with tc.high_priority(offset=LOW):
                proj_qk(3, "q")
            attn_qblock(0, 3)
            attn_qblock(1, 0)
            with tc.high_priority(offset=LOW):
                proj_qk(5, "q")
            attn_qblock(1, 1)
            with tc.high_priority(offset=LOW):
                proj_qk(6, "q")
            attn_qblock(1, 2)
            with tc.high_priority(offset=LOW):
                proj_qk(7, "q")
            attn_qblock(1, 3)

    nc.compile()
    return nc


def get_nc():
    if "nc" not in _NC_CACHE:
        _NC_CACHE["nc"] = build_nc()
    return _NC_CACHE["nc"]


def make_in_maps(seq_input, WQ, bQ, WK, bK, WV, bV):
    x = np.asarray(seq_input, dtype=np.float32).reshape(NTOK, D)
    xt = np.ascontiguousarray(x.T).astype(ml_dtypes.bfloat16)

    def swizzle(w):
        # [D, FPC] -> [128, NCH*FPC]: row p, col c*FPC+j = w[c*128+p, j]
        return np.ascontiguousarray(
            w.reshape(NCH, 128, FPC).transpose(1, 0, 2).reshape(128, NCH * FPC)
        ).astype(ml_dtypes.bfloat16)

    in_maps = []
    for c in range(NCORES):
        lo, hi = c * FPC, (c + 1) * FPC
        in_maps.append(
            {
                "xt": xt,
                "wq": swizzle(WQ[:, lo:hi]),
                "wk": swizzle(WK[:, lo:hi]),
                "wv": swizzle(WV[:, lo:hi]),
                "bq": np.ascontiguousarray(bQ[lo:hi]).reshape(FPC, 1).astype(np.float32),
            }
        )
    return in_maps


def assemble(res, bV):
    """res -> full [B, S, H*HS] output; applies the host-side V bias."""
    parts = []
    for c in range(NCORES):
        o = res.results[c]["out"]  # [130, 4096] unnormalized, feature-major
        lo = c * FPC
        for h in range(2):
            num = o[h * (HS + 1) : h * (HS + 1) + HS, :]      # [64, 4096]
            den = o[h * (HS + 1) + HS, :]                     # [4096]
            parts.append((num / den).T + bV[lo + h * HS : lo + (h + 1) * HS])
    full = np.concatenate(parts, axis=1)  # [4096, 1024]
    return full.reshape(B, S, H * HS)


def run(in_maps, trace=False):
    nc = get_nc()
    return bass_utils.run_bass_kernel_spmd(nc, in_maps, core_ids=list(range(NCORES)), trace=trace)


def kernel(seq_input, WQ, bQ, WK, bK, WV, bV):
    in_maps = make_in_maps(
        np.asarray(seq_input, np.float32),
        np.asarray(WQ, np.float32), np.asarray(bQ, np.float32),
        np.asarray(WK, np.float32), np.asarray(bK, np.float32),
        np.asarray(WV, np.float32), np.asarray(bV, np.float32),
    )
    res = run(in_maps)
    return assemble(res, np.asarray(bV, np.float32))


# revision 17
# speedup vs baseline: 1.0471x; 1.0159x over previous
"""Multi-head attention Trainium2 Bass kernel (v2).

Problem: B=2, S=2048, D=1024, H=16, HS=64.
Sharding: tensor-parallel over heads — each of 8 cores computes 2 heads
(128 contiguous output-feature columns) for both batches; host concatenates.

v2 design (vs the v1 phase-separated kernel):
  * The scalar-engine exp stream is the roofline: 16.8M elements/core at
    1 elem/lane/cycle @1.2GHz = ~110us, ~147us with the 352-cycle
    per-ACTIVATE overhead at N=1024.  Everything else hides under it.
  * q-blocks of 512: per (batch, qblock, key-chunk) one PSUM tile
    [128 keys, 1024] holds both heads' sims side by side, so one ACTIVATE
    covers both heads.  PSUM: sim 2x2 banks + pvp 2x1 + proj 2x1 = 8.
  * The two sim matmuls (K=64 each) go to disjoint PE row groups
    (h0 rows 0-63, h1 rows 64-127) emitted back-to-back so they can run
    concurrently in the array.
  * No bias matmuls at all: bK is dropped (constant per q column ->
    cancels in softmax), bQ is added by the DVE during the Q psum->sbuf
    copy (per-partition scalar add), bV is applied by the host after the
    final divide (P@(v+b)/P@1 = P@v/P@1 + b).
  * Projections are emitted after the attention loops at lower scheduler
    priority; the Tile scheduler runs them in PE gaps under the exp
    stream.  Only the first q-block's K/V/Q prefix runs up front.
  * V' is token-major [tok, 64v+1] per head; the softmax-denominator ones
    columns are pre-memset into SBUF (stride-65 pattern) instead of being
    produced by a bias matmul.
"""

import sys

sys.path.insert(0, "/opt/trn_rl_repo")

import ml_dtypes
import numpy as np

import concourse.bass as bass
import concourse.mybir as mybir
import concourse.tile as tile
from concourse import bacc
from concourse import bass_utils

B, S, D = 2, 2048, 1024
H, HS = 16, 64
NCORES = 8
NTOK = B * S                  # 4096
FPC = (H // NCORES) * HS      # 128 output-feature cols per core (2 heads)
TT = 512                      # token tile for projections
NTT = NTOK // TT              # 8
NCH = D // 128                # 8 contraction chunks
QB = 512                      # q block width
NQB = S // QB                 # 4 q blocks per batch
KT = 128                      # key chunk in attention
NKT = S // KT                 # 16
VW = 2 * (HS + 1)             # 130: [V_h0 | 1 | V_h1 | 1] columns per chunk

F32 = mybir.dt.float32
BF16 = mybir.dt.bfloat16
I16 = mybir.dt.int16

# Schraudolph exp on the DVE, int16 form: bf16bits(exp(sim/8)) ~=
# round(sim * 2^7*log2e/8 + (127*2^7 - C16 - 0.5)).  C16 = 366393/65536
# (the standard rms-optimal int32 constant scaled to the bf16 grid); the
# -0.5 converts round-to-nearest into the truncation the constant was
# tuned for.  Every 4th key-chunk uses this path, freeing the scalar
# engine; measured end-to-end rms error ~5.3e-3 (gate 2e-2).
SCH_A = 128.0 * 1.4426950408889634 / 8.0
SCH_B = 127.0 * 128.0 - 366393.0 / 65536.0 - 0.5
# Schraudolph currently disabled: with PE ~= ACT per key-chunk, moving exp
# to the DVE fragments the stream (strict-FIFO head-of-line blocking)
# without lowering the wall clock.  Set to 4 to re-enable (rms ~5.9e-3).
SCH_EVERY = 10**9

_NC_CACHE = {}


def build_nc():
    nc = bacc.Bacc("TRN2", target_bir_lowering=False, debug=False, num_devices=NCORES)
    xt = nc.dram_tensor("xt", [D, NTOK], BF16, kind="ExternalInput").ap()
    # weights ship pre-swizzled to the SBUF layout [128, chunk*feat]
    wq = nc.dram_tensor("wq", [128, NCH * FPC], BF16, kind="ExternalInput").ap()
    wk = nc.dram_tensor("wk", [128, NCH * FPC], BF16, kind="ExternalInput").ap()
    wv = nc.dram_tensor("wv", [128, NCH * FPC], BF16, kind="ExternalInput").ap()
    bq = nc.dram_tensor("bq", [FPC, 1], F32, kind="ExternalInput").ap()
    out = nc.dram_tensor("out", [2 * (HS + 1), NTOK], F32, kind="ExternalOutput").ap()

    with tile.TileContext(nc) as tc:
        with (
            tc.tile_pool(name="persist", bufs=1) as pp,
            tc.tile_pool(name="work", bufs=2) as wkp,
            tc.tile_pool(name="psA", bufs=2, space="PSUM") as psA,
            tc.tile_pool(name="psB", bufs=2, space="PSUM") as psB,
        ):
            # ---------------- persistent SBUF ------------------------------
            xtc = [pp.tile([128, NTOK], BF16, name=f"xt_{c}") for c in range(NCH)]
            wq_sb = pp.tile([128, NCH * FPC], BF16)
            wk_sb = pp.tile([128, NCH * FPC], BF16)
            wv_sb = pp.tile([128, NCH * FPC], BF16)
            bq_sb = pp.tile([128, 1], F32)
            qt_sb = pp.tile([128, NTOK], BF16)   # Q^T + bQ: [feat(2 heads), tok]
            kt_sb = pp.tile([128, NTOK], BF16)   # K^T (no bias)
            vp_sb = pp.tile([128, (NTOK // 128) * VW], BF16)  # V' [tok128 chunks, 65+65]

            # DMA order = first-exp critical path: K weights, then t0 X
            # slices, then the rest in increasing-deadline order with
            # progressively larger (faster) transfers.
            nc.sync.dma_start(bq_sb[:], bq[:, :])
            nc.sync.dma_start(wk_sb[:], wk[:, :])
            for c in range(NCH):
                nc.sync.dma_start(xtc[c][:, 0:TT], xt[c * 128 : (c + 1) * 128, 0:TT])
            nc.sync.dma_start(wq_sb[:], wq[:, :])
            nc.sync.dma_start(wv_sb[:], wv[:, :])
            for c in range(NCH):
                nc.sync.dma_start(xtc[c][:, TT : 2 * TT], xt[c * 128 : (c + 1) * 128, TT : 2 * TT])
            for c in range(NCH):
                nc.sync.dma_start(xtc[c][:, 2 * TT : 4 * TT], xt[c * 128 : (c + 1) * 128, 2 * TT : 4 * TT])
            for c in range(NCH):
                nc.sync.dma_start(xtc[c][:, S : 2 * S], xt[c * 128 : (c + 1) * 128, S : 2 * S])

            # Dummy matmuls on never-written SBUF warm the PE clock gate
            # (HAM) during the DMA prefix so the real prefix runs at 2.4GHz.
            warm_in = pp.tile([64, TT], BF16, name="warm_in")
            nc.vector.memset(warm_in[:], 0.0)
            for w in range(10):
                wps = psA.tile([128, TT], F32, name=f"warm_{w}", tag="prj", padded_shape=[128, TT])
                nc.tensor.matmul(wps[:], warm_in[:, 0:128], warm_in[:], start=True, stop=True)

            # Softmax-denominator ones columns: cols 64, 129, 194, ... (64+65k).
            nc.vector.memset(vp_sb[:, HS :: HS + 1], 1.0)

            # ---------------- projection building blocks -------------------
            def proj_qk(t, which):
                """Project Q or K for t-tile t (512 tokens), feature-major."""
                tsl = slice(t * TT, (t + 1) * TT)
                w_sb, dst = (wq_sb, qt_sb) if which == "q" else (wk_sb, kt_sb)
                pj = psA.tile([128, TT], F32, name=f"pj_{which}_{t}", tag="prj", padded_shape=[128, TT])
                for c in range(NCH):
                    nc.tensor.matmul(
                        pj[:], w_sb[:, c * FPC : (c + 1) * FPC], xtc[c][:, tsl],
                        start=(c == 0), stop=(c == NCH - 1),
                    )
                if which == "q":
                    nc.vector.tensor_scalar_add(dst[:, tsl], pj[:], bq_sb[:])
                else:
                    nc.vector.tensor_copy(dst[:, tsl], pj[:])

            def proj_v(ch):
                """Project V' token-major for 128-token chunk ch."""
                psv = psA.tile([128, FPC], F32, name=f"pv_{ch}", tag="prj", padded_shape=[128, TT])
                for c in range(NCH):
                    nc.tensor.matmul(
                        psv[:], xtc[c][:, ch * 128 : (ch + 1) * 128],
                        wv_sb[:, c * FPC : (c + 1) * FPC],
                        start=(c == 0), stop=(c == NCH - 1),
                    )
                base = ch * VW
                for h in range(2):
                    nc.vector.tensor_copy(
                        vp_sb[:, base + h * (HS + 1) : base + h * (HS + 1) + HS],
                        psv[:, h * HS : (h + 1) * HS],
                    )

            # ---------------- attention ------------------------------------
            def attn_qblock(b, qb, weave=None):
                """weave: {kt: [thunk, ...]} — projection quanta emitted after
                iteration kt, small enough (<~1.8us of PE) to hide in the
                scalar engine's pipeline backlog."""
                qsl = slice(b * S + qb * QB, b * S + (qb + 1) * QB)
                pvp = [
                    psB.tile([HS + 1, QB], F32, name=f"pvp_{b}_{qb}_{h}", tag="pvp", padded_shape=[128, QB])
                    for h in range(2)
                ]
                for kt in range(NKT):
                    ksl = slice(b * S + kt * KT, b * S + (kt + 1) * KT)
                    ch = b * NKT + kt
                    simt = psA.tile([128, 2 * QB], F32, name=f"sim_{b}_{qb}_{kt}", tag="sim", padded_shape=[128, 2 * QB])
                    # two K=64 matmuls into disjoint PE row groups, adjacent
                    # in the queue so they can run concurrently
                    for h in range(2):
                        hp = h * HS
                        nc.tensor.matmul(
                            simt[:, h * QB : (h + 1) * QB],
                            kt_sb[hp : hp + HS, ksl],
                            qt_sb[hp : hp + HS, qsl],
                            start=True, stop=True,
                            tile_position=(hp, 0),
                        )
                    pt = wkp.tile([128, 2 * QB], BF16, name=f"pt_{b}_{qb}_{kt}", tag="pt", bufs=3)
                    if SCH_EVERY <= NKT and kt % SCH_EVERY == 1:
                        # Schraudolph exp on the DVE (int16 bit-trick),
                        # freeing the scalar engine for this chunk.
                        nc.vector.tensor_scalar(
                            pt.bitcast(I16)[:], simt[:], SCH_A, SCH_B,
                            op0=mybir.AluOpType.mult, op1=mybir.AluOpType.add,
                        )
                    else:
                        nc.scalar.activation(pt[:], simt[:], mybir.ActivationFunctionType.Exp, scale=1.0 / np.sqrt(HS))
                    for h in range(2):
                        nc.tensor.matmul(
                            pvp[h][:],
                            vp_sb[:, ch * VW + h * (HS + 1) : ch * VW + (h + 1) * (HS + 1)],
                            pt[:, h * QB : (h + 1) * QB],
                            start=(kt == 0), stop=(kt == NKT - 1),
                        )
                    if weave and kt in weave:
                        for thunk in weave[kt]:
                            thunk()
                for h in range(2):
                    ot = wkp.tile([HS + 1, QB], F32, name=f"ot_{b}_{qb}_{h}", tag="ot", bufs=4)
                    nc.vector.tensor_copy(ot[:], pvp[h][:])
                    nc.sync.dma_start(
                        out[h * (HS + 1) : (h + 1) * (HS + 1), qsl], ot[:]
                    )

            # ---------------- schedule -------------------------------------
            # Projections are woven between attention iterations as small
            # normal-priority quanta; each quantum hides inside the scalar
            # engine's pipeline backlog.  Deadlines: sim(kt) needs its K
            # t-tile by ~kt-1, PV(kt) needs V' chunk kt (can lag ~2 kts via
            # pt buffering), Q t-tile needed at its q-block start.
            def K_(t):
                return lambda: proj_qk(t, "k")

            def Q_(t):
                return lambda: proj_qk(t, "q")

            def V_(*chs):
                return lambda: [proj_v(ch) for ch in chs]

            proj_qk(0, "k")
            proj_qk(0, "q")
            proj_v(0)
            proj_v(1)
            attn_qblock(0, 0, {
                0: [K_(1)], 1: [V_(2, 3)], 3: [V_(4, 5)], 4: [K_(2)],
                5: [V_(6, 7)], 7: [V_(8, 9)], 8: [K_(3)], 9: [V_(10, 11)],
                11: [V_(12, 13)], 13: [V_(14, 15)], 14: [Q_(1)],
            })
            attn_qblock(0, 1, {
                0: [K_(4)], 1: [V_(16, 17)], 3: [V_(18, 19)], 5: [K_(5)],
                7: [V_(20, 21)], 9: [V_(22, 23)], 11: [K_(6)], 13: [Q_(2)],
            })
            attn_qblock(0, 2, {
                0: [V_(24, 25)], 2: [V_(26, 27)], 4: [K_(7)], 6: [V_(28, 29)],
                8: [V_(30, 31)], 11: [Q_(4)], 13: [Q_(3)],
            })
            attn_qblock(0, 3, {
                3: [Q_(5)], 8: [Q_(6)],
            })
            attn_qblock(1, 0, {
                5: [Q_(7)],
            })
            attn_qblock(1, 1)
            attn_qblock(1, 2)
            attn_qblock(1, 3)

    nc.compile()
    return nc


def get_nc():
    if "nc" not in _NC_CACHE:
        _NC_CACHE["nc"] = build_nc()
    return _NC_CACHE["nc"]


def make_in_maps(seq_input, WQ, bQ, WK, bK, WV, bV):
    x = np.asarray(seq_input, dtype=np.float32).reshape(NTOK, D)
    xt = np.ascontiguousarray(x.T).astype(ml_dtypes.bfloat16)

    def swizzle(w):
        # [D, FPC] -> [128, NCH*FPC]: row p, col c*FPC+j = w[c*128+p, j]
        return np.ascontiguousarray(
            w.reshape(NCH, 128, FPC).transpose(1, 0, 2).reshape(128, NCH * FPC)
        ).astype(ml_dtypes.bfloat16)

    in_maps = []
    for c in range(NCORES):
        lo, hi = c * FPC, (c + 1) * FPC
        in_maps.append(
            {
                "xt": xt,
                "wq": swizzle(WQ[:, lo:hi]),
                "wk": swizzle(WK[:, lo:hi]),
                "wv": swizzle(WV[:, lo:hi]),
                "bq": np.ascontiguousarray(bQ[lo:hi]).reshape(FPC, 1).astype(np.float32),
            }
        )
    return in_maps


def assemble(res, bV):
    """res -> full [B, S, H*HS] output; applies the host-side V bias."""
    parts = []
    for c in range(NCORES):
        o = res.results[c]["out"]  # [130, 4096] unnormalized, feature-major
        lo = c * FPC
        for h in range(2):
            num = o[h * (HS + 1) : h * (HS + 1) + HS, :]      # [64, 4096]
            den = o[h * (HS + 1) + HS, :]                     # [4096]
            parts.append((num / den).T + bV[lo + h * HS : lo + (h + 1) * HS])
    full = np.concatenate(parts, axis=1)  # [4096, 1024]
    return full.reshape(B, S, H * HS)


def run(in_maps, trace=False):
    nc = get_nc()
    return bass_utils.run_bass_kernel_spmd(nc, in_maps, core_ids=list(range(NCORES)), trace=trace)


def kernel(seq_input, WQ, bQ, WK, bK, WV, bV):
    in_maps = make_in_maps(
        np.asarray(seq_input, np.float32),
        np.asarray(WQ, np.float32), np.asarray(bQ, np.float32),
        np.asarray(WK, np.float32), np.asarray(bK, np.float32),
        np.asarray(WV, np.float32), np.asarray(bV, np.float32),
    )
    res = run(in_maps)
    return assemble(res, np.asarray(bV, np.float32))
